# revision 18
# baseline (speedup 1.0000x reference)
"""Trainium2 Bass kernel for nn_AttentionBlock (B=16, C=512, H=W=64, 8 heads).

Channel-attention block: GroupNorm(8 groups) -> 1x1 qkv -> scores over
channel dims (contract spatial N=4096) -> softmax -> att @ v -> 1x1 out
projection -> residual.

Sharding: data-parallel over batch. 16 batches / 8 cores = 2 per core.
No collectives. Each core runs the identical program on its 2 batches.

v2 layout/scheduling notes:
  x     [C, N] fp32, 4 channel-chunk tiles of [128, 4096]
  h     (groupnorm output) same layout, bf16
  q,k   [N, 2C] orientation (spatial on partitions), bf16, transient tiles
  v     [C, N] bf16, resident
  scores head-pairs: 4 matmuls per spatial chunk with 128-wide stationary
        (2 heads), all into one [128, 512] psum tile; the off-diagonal
        quadrants are garbage that softmax ignores.
  attT  4 block-diagonal [128,128] bf16 tiles (heads 2p, 2p+1), so att@v
        runs one full-partition matmul per (chunk, t-block) -- half the
        cycles of quadrant-split 64-partition matmuls.
  out   = w_out @ hv + (w_out @ (att @ b_v) + b_out) + x   (residual)

Engine split: PE matmuls; DVE bn_stats + softmax + final residual STT;
ACT psum evacuations (qk, v, hv+bias); Pool normalize + qk bias adds.
DMA: x loads on sync, xr reloads on scalar ring, stores on sync.
Batch 1's stats/normalize are overlapped with batch 0's qkv/att phases.
"""

import numpy as np
import ml_dtypes

import concourse.bacc as bacc
import concourse.tile as tile
from concourse import mybir
from concourse.bass_utils import run_bass_kernel_spmd
from concourse.masks import make_identity

BF = mybir.dt.bfloat16
F32 = mybir.dt.float32
AX = mybir.AxisListType
OP = mybir.AluOpType
AF = mybir.ActivationFunctionType

C = 512
NH = 8
D = 64  # head dim
G = 8   # groupnorm groups
CK = C // 128  # 4 channel chunks
EPS = 1e-5
N_CORES = 8


def build_program(B=2, N=4096, debug=False):
    SP = N // 128   # spatial chunks for qk/scores
    NT = N // 512   # 512-col tiles
    SUB = N // 512  # bn_stats subgroups (free dim <= 512)
    scale = float(1.0 / np.sqrt(D))
    LAG = 3         # score-emission lag behind qk chunks (ACT->Pool evac)
    PRE = 3         # batch-(b+1) qk chunks stashed during batch-b softmax

    nc = bacc.Bacc("TRN2", target_bir_lowering=False, debug=debug,
                   num_devices=N_CORES)

    x_d = nc.dram_tensor("x", [B, C, N], F32, kind="ExternalInput")
    wqk_d = nc.dram_tensor("wqkT", [C, 2 * C], BF, kind="ExternalInput")
    wv_d = nc.dram_tensor("wvR", [C, C], BF, kind="ExternalInput")
    wo_d = nc.dram_tensor("woT", [C, C], BF, kind="ExternalInput")
    bqk_d = nc.dram_tensor("bqk", [1, 2 * C], BF, kind="ExternalInput")
    bv_d = nc.dram_tensor("bv", [C, 1], BF, kind="ExternalInput")
    bo_d = nc.dram_tensor("bo", [C, 1], F32, kind="ExternalInput")
    gam_d = nc.dram_tensor("gamma", [C, 1], F32, kind="ExternalInput")
    bet_d = nc.dram_tensor("beta", [C, 1], F32, kind="ExternalInput")
    indf_d = nc.dram_tensor("indf", [C, G], F32, kind="ExternalInput")
    indb_d = nc.dram_tensor("indb", [G, C], F32, kind="ExternalInput")
    out_d = nc.dram_tensor("out", [B, C, N], F32, kind="ExternalOutput")

    with tile.TileContext(nc) as tc:
        import contextlib
        ctx = contextlib.ExitStack()
        with ctx:
            persist = ctx.enter_context(tc.tile_pool(name="persist", bufs=1))
            big = ctx.enter_context(tc.tile_pool(name="big", bufs=1))
            mid = ctx.enter_context(tc.tile_pool(name="mid", bufs=3))
            small = ctx.enter_context(tc.tile_pool(name="small", bufs=1))
            # pool A: qk projection halves (qkv phase) + out-proj (att phase)
            # pool B: v-proj, hv, transposes, groupnorm matmuls
            ps_a = ctx.enter_context(
                tc.tile_pool(name="ps_a", bufs=4, space="PSUM"))
            ps_sc = ctx.enter_context(
                tc.tile_pool(name="ps_sc", bufs=1, space="PSUM"))
            ps_b = ctx.enter_context(
                tc.tile_pool(name="ps_b", bufs=3, space="PSUM"))

            # ---- persistent: weights / constants ----
            wqk = []
            wv = []
            wo = []
            bv_sb = []
            bo_sb = []
            gam = []
            bet = []
            for k in range(CK):
                t = persist.tile([128, 2 * C], BF, tag=f"wqk{k}")
                nc.gpsimd.dma_start(out=t, in_=wqk_d.ap()[k * 128:(k + 1) * 128, :])
                wqk.append(t)
                t = persist.tile([128, C], BF, tag=f"wv{k}")
                nc.gpsimd.dma_start(out=t, in_=wv_d.ap()[k * 128:(k + 1) * 128, :])
                wv.append(t)  # raw wv rows [e-chunk, cin] for w' = att @ wv
                t = persist.tile([128, C], BF, tag=f"wo{k}")
                nc.gpsimd.dma_start(out=t, in_=wo_d.ap()[k * 128:(k + 1) * 128, :])
                wo.append(t)
                t = persist.tile([128, 1], BF, tag=f"bv{k}")
                nc.gpsimd.dma_start(out=t, in_=bv_d.ap()[k * 128:(k + 1) * 128, :])
                bv_sb.append(t)
                t = persist.tile([128, 1], F32, tag=f"bo{k}")
                nc.gpsimd.dma_start(out=t, in_=bo_d.ap()[k * 128:(k + 1) * 128, :])
                bo_sb.append(t)
                t = persist.tile([128, 1], F32, tag=f"gam{k}")
                nc.gpsimd.dma_start(out=t, in_=gam_d.ap()[k * 128:(k + 1) * 128, :])
                gam.append(t)
                t = persist.tile([128, 1], F32, tag=f"bet{k}")
                nc.gpsimd.dma_start(out=t, in_=bet_d.ap()[k * 128:(k + 1) * 128, :])
                bet.append(t)
            # q/k bias replicated across all 128 partitions (spatial rows)
            import concourse.bass as bass
            bqk_rep = persist.tile([128, 2 * C], BF, tag="bqk_rep")
            _bqk_ap = bqk_d.ap()
            nc.gpsimd.dma_start(
                out=bqk_rep,
                in_=bass.AP(tensor=_bqk_ap.tensor, offset=_bqk_ap.offset,
                            ap=[[0, 128], [1, 2 * C]]))

            zero1 = persist.tile([1, 128], BF, tag="zero1")
            nc.gpsimd.memset(zero1, 0.0)
            zrhs512 = persist.tile([1, 512], BF, tag="zrhs512")
            nc.gpsimd.memset(zrhs512, 0.0)
            ident = persist.tile([128, 128], BF, tag="ident")
            make_identity(nc, ident)
            eps_t = persist.tile([128, 1], F32, tag="eps")
            nc.gpsimd.memset(eps_t, EPS)
            # block-diagonal att tiles: off-diagonal quadrants stay zero
            att_bf = []
            for p in range(CK):
                t = persist.tile([128, 128], BF, tag=f"attbf{p}")
                nc.gpsimd.memset(t, 0.0)
                att_bf.append(t)
            # group indicator matrices (groupnorm cross-partition reduce)
            indf = []
            for k in range(CK):
                t = persist.tile([128, G], F32, tag=f"indf{k}")
                nc.gpsimd.dma_start(
                    out=t, in_=indf_d.ap()[k * 128:(k + 1) * 128, :])
                indf.append(t)
            indb = persist.tile([G, C], F32, tag="indb")
            nc.gpsimd.dma_start(out=indb, in_=indb_d.ap())

            # ---- building blocks ----
            NP = N // 1024  # x pieces per chunk

            def load_x(b):
                # per-piece tiles so batch b+1 pieces can recycle as soon as
                # the prior batch's readers of that piece are done; DMAs are
                # emitted piece-major so a late piece never head-of-line
                # blocks another chunk's earlier piece on the sync ring
                xs = [[None] * NP for _ in range(CK)]
                for p in range(NP):
                    for k in range(CK):
                        t = big.tile([128, 1024], F32, tag=f"x{k}p{p}",
                                     name=f"x{k}p{p}")
                        nc.sync.dma_start(
                            out=t,
                            in_=x_d.ap()[b, k * 128:(k + 1) * 128,
                                         p * 1024:(p + 1) * 1024])
                        xs[k][p] = t
                return xs

            def bn_alloc():
                sts = []
                for k in range(CK):
                    sts.append(small.tile([128, SUB, 6], F32, tag=f"st{k}",
                                          name=f"st{k}"))
                return sts

            def bn_piece(xs, sts, k, j):
                nc.vector.bn_stats(
                    out=sts[k][:, j, :],
                    in_=xs[k][j // 2][:, (j % 2) * 512:(j % 2) * 512 + 512])

            def bn_finish(sts):
                mvs = []
                for k in range(CK):
                    mv = small.tile([128, 2], F32, tag=f"mv{k}")
                    nc.vector.bn_aggr(out=mv, in_=sts[k])
                    mvs.append(mv)
                # rhs2: col0 = mean_p, col1 = mean_p^2 + var_p = E[x^2]_p
                rhs2s = []
                for k in range(CK):
                    r2 = small.tile([128, 2], F32, tag=f"r2{k}")
                    nc.gpsimd.tensor_copy(out=r2[:, 0:1], in_=mvs[k][:, 0:1])
                    nc.vector.scalar_tensor_tensor(
                        out=r2[:, 1:2], in0=mvs[k][:, 0:1],
                        scalar=mvs[k][:, 0:1], in1=mvs[k][:, 1:2],
                        op0=OP.mult, op1=OP.add)
                    rhs2s.append(r2)
                # cross-partition reduce to per-group stats [8, 2]
                pg = ps_b.tile([G, 2], F32, tag="pb")
                for k in range(CK):
                    nc.tensor.matmul(pg, indf[k], rhs2s[k],
                                     start=(k == 0), stop=(k == CK - 1))
                sg = small.tile([G, 2], F32, tag="sg")
                nc.vector.tensor_copy(out=sg, in_=pg)
                t2 = small.tile([G, 1], F32, tag="t2")
                nc.vector.tensor_mul(out=t2, in0=sg[:, 0:1], in1=sg[:, 0:1])
                vs = small.tile([G, 1], F32, tag="vs")
                nc.vector.tensor_sub(out=vs, in0=sg[:, 1:2], in1=t2)
                # rstd = exp(-0.5 * ln(var + eps)); Ln/Exp share a table set
                lnv = small.tile([G, 1], F32, tag="lnv")
                nc.scalar.activation(out=lnv, in_=vs, func=AF.Ln,
                                     bias=eps_t[0:G, :], scale=1.0)
                rstd = small.tile([G, 1], F32, tag="rstd")
                nc.scalar.activation(out=rstd, in_=lnv, func=AF.Exp, scale=-0.5)
                bcr = small.tile([G, 2], F32, tag="bcr")
                nc.gpsimd.tensor_copy(out=bcr[:, 0:1], in_=sg[:, 0:1])
                nc.gpsimd.tensor_copy(out=bcr[:, 1:2], in_=rstd)
                # broadcast group stats back to channels; affine coeffs
                scs = []
                nbs = []
                for k in range(CK):
                    pbc = ps_b.tile([128, 2], F32, tag="pb")
                    nc.tensor.matmul(pbc, indb[:, k * 128:(k + 1) * 128], bcr,
                                     start=True, stop=True)
                    sc = small.tile([128, 1], F32, tag=f"sc{k}")
                    nc.vector.tensor_mul(out=sc, in0=pbc[:, 1:2], in1=gam[k])
                    t4 = small.tile([128, 1], F32, tag=f"t4{k}")
                    nc.vector.tensor_scalar_mul(out=t4, in0=pbc[:, 0:1],
                                                scalar1=sc)
                    nb = small.tile([128, 1], F32, tag=f"nb{k}")
                    nc.vector.tensor_sub(out=nb, in0=bet[k], in1=t4)
                    scs.append(sc)
                    nbs.append(nb)
                return scs, nbs

            def h_alloc(par):
                # blocks 2..7 only (cols 1024:4096); blocks 0-1 live in hpre
                return [big.tile([128, N - 1024], BF, tag=f"h{k}b{par}",
                                 name=f"h{k}")
                        for k in range(CK)]

            def normalize_block(hs, xs, coeffs, t):
                # ACT: out = Identity(scale*x + bias), per-partition APs.
                # h blocks 0-1 live in the small hpre tiles (cols 0:1024);
                # hs tiles carry cols 1024:4096 only (offset by 1024).
                scs, nbs = coeffs
                xsl = slice((t % 2) * 512, (t % 2) * 512 + 512)
                sl = slice(t * 512 - 1024, (t + 1) * 512 - 1024)
                for k in range(CK):
                    nc.scalar.activation(
                        out=hs[k][:, sl], in_=xs[k][t // 2][:, xsl],
                        func=AF.Identity, bias=nbs[k], scale=scs[k])

            def hpre_alloc(par):
                return [big.tile([128, 1024], BF, tag=f"hpre{k}b{par}",
                                 name=f"hpre{k}")
                        for k in range(CK)]

            def hpre_ops(hp, xs, coeffs):
                # 8 thunks: normalize blocks 0-1 into the hpre tiles
                scs, nbs = coeffs
                ops = []
                for t in range(2):
                    xsl = slice((t % 2) * 512, (t % 2) * 512 + 512)
                    sl = slice(t * 512, (t + 1) * 512)
                    for k in range(CK):
                        def op(t=t, k=k, xsl=xsl, sl=sl):
                            nc.scalar.activation(
                                out=hp[k][:, sl], in_=xs[k][0][:, xsl],
                                func=AF.Identity, bias=nbs[k], scale=scs[k])
                        ops.append(op)
                return ops

            def normalize_pre(hp, xs, coeffs):
                for op in hpre_ops(hp, xs, coeffs):
                    op()

            def h_src(hp, hs, k, c0, c1):
                if c1 <= 1024:
                    return hp[k][:, c0:c1]
                return hs[k][:, c0 - 1024:c1 - 1024]

            def scores_setup():
                Tsc = ps_sc.tile([128, 512], F32, tag="sc")
                nc.tensor.matmul(Tsc, zero1, zrhs512, start=True, stop=False,
                                 skip_group_check=True)
                return Tsc

            def v_alloc():
                return [big.tile([128, N], BF, tag=f"v{k}", name=f"v{k}")
                        for k in range(CK)]

            def qk_chunk(hp, hs, s):
                # qk projection for one 128-row spatial chunk; evac on ACT,
                # bias add on Pool (scores consume LAG chunks later)
                qk = mid.tile([128, 2 * C], BF, tag="qk", bufs=5)
                for half in range(2):
                    pqk = ps_a.tile([128, 512], F32, tag="pa")
                    wseg = slice(half * 512, (half + 1) * 512)
                    for k in range(CK):
                        nc.tensor.matmul(
                            pqk, h_src(hp, hs, k, s * 128, (s + 1) * 128),
                            wqk[k][:, wseg], start=(k == 0),
                            stop=(k == CK - 1))
                    if half == 0:
                        # fused evac + bias add in one DVE pass
                        nc.vector.tensor_add(out=qk[:, wseg], in0=pqk,
                                             in1=bqk_rep[:, wseg])
                    else:
                        nc.scalar.copy(out=qk[:, wseg], in_=pqk)
                        nc.gpsimd.tensor_add(out=qk[:, wseg],
                                             in0=qk[:, wseg],
                                             in1=bqk_rep[:, wseg])
                return qk

            def emit_scores(qk, Tsc):
                # head-pair matmuls: stationary = 2 heads of q (128 cols);
                # diagonal 64x64 blocks of each [128,128] slot are the real
                # per-head scores, off-diagonal is ignored garbage.
                for p in range(CK):
                    nc.tensor.matmul(
                        Tsc[:, p * 128:(p + 1) * 128],
                        qk[:, p * 128:(p + 1) * 128],
                        qk[:, 512 + p * 128:512 + (p + 1) * 128],
                        start=False, stop=False, skip_group_check=True)

            def softmax(Tsc):
                # softmax without max-subtraction: logits = S/8 are bounded
                # well inside fp32 exp range for this distribution.
                p_f = small.tile([128, 512], F32, tag="pf")
                nc.scalar.activation(out=p_f, in_=Tsc, func=AF.Exp,
                                     scale=scale)
                rsum = small.tile([128, CK], F32, tag="rsum")
                for p in range(CK):
                    c0 = p * 128
                    nc.vector.reduce_sum(
                        out=rsum[0:64, p:p + 1],
                        in_=p_f[0:64, c0:c0 + 64], axis=AX.X)
                    nc.vector.reduce_sum(
                        out=rsum[64:128, p:p + 1],
                        in_=p_f[64:128, c0 + 64:c0 + 128], axis=AX.X)
                rinv = small.tile([128, CK], F32, tag="rinv")
                nc.vector.reciprocal(out=rinv, in_=rsum)
                for p in range(CK):
                    c0 = p * 128
                    nc.vector.tensor_scalar_mul(
                        out=att_bf[p][0:64, 0:64],
                        in0=p_f[0:64, c0:c0 + 64],
                        scalar1=rinv[0:64, p:p + 1])
                    nc.vector.tensor_scalar_mul(
                        out=att_bf[p][64:128, 64:128],
                        in0=p_f[64:128, c0 + 64:c0 + 128],
                        scalar1=rinv[64:128, p:p + 1])
                aTs = []
                for p in range(CK):
                    ptr = ps_b.tile([128, 128], BF, tag="pb")
                    nc.tensor.transpose(ptr, att_bf[p], ident)
                    aT = small.tile([128, 128], BF, tag=f"aT{p}")
                    nc.vector.tensor_copy(out=aT, in_=ptr)
                    aTs.append(aT)
                # c = att @ b_v, later folded into the composite bias
                csb = []
                for k in range(CK):
                    pcv = ps_b.tile([128, 1], F32, tag="pb")
                    nc.tensor.matmul(pcv, aTs[k], bv_sb[k],
                                     start=True, stop=True)
                    ct = small.tile([128, 1], BF, tag=f"c{k}")
                    nc.vector.tensor_copy(out=ct, in_=pcv)
                    csb.append(ct)
                return aTs, csb

            def w_chain(aTs, csb):
                # Reassociated attention: out = (wo @ blockdiag(att) @ wv) @ h.
                # The composite weight is tiny ([C, C]) vs applying att over
                # N=4096 columns, so v-projection, att@v and both their
                # evacuations collapse into one weight-build + one apply pass.
                # w' = blockdiag(att) @ wv : [cout-block j, cin]
                wps = []
                for j in range(CK):
                    pw = ps_b.tile([128, 512], F32, tag="pb")
                    nc.tensor.matmul(pw, aTs[j], wv[j], start=True, stop=True)
                    wp = small.tile([128, 512], BF, tag=f"wp{j}")
                    nc.vector.tensor_copy(out=wp, in_=pw)
                    wps.append(wp)
                # w'' = wo @ w' : [oout-block m, cin]
                w2s = []
                for m in range(CK):
                    pw2 = ps_b.tile([128, 512], F32, tag="pb")
                    for j in range(CK):
                        nc.tensor.matmul(
                            pw2, wo[j][:, m * 128:(m + 1) * 128], wps[j],
                            start=(j == 0), stop=(j == CK - 1))
                    w2 = small.tile([128, 512], BF, tag=f"w2{m}")
                    nc.scalar.copy(out=w2, in_=pw2)
                    w2s.append(w2)
                # w''T for use as matmul stationary: [cin-chunk k, oout].
                # 4 transposes land in one psum tile; evacs alternate DVE/ACT
                w2Ts = []
                for k in range(CK):
                    pt = ps_b.tile([128, 512], BF, tag="pb")
                    for m in range(CK):
                        nc.tensor.matmul(
                            pt[:, m * 128:(m + 1) * 128],
                            w2s[m][:, k * 128:(k + 1) * 128], ident,
                            is_transpose=True, start=(m == 0),
                            stop=(m == CK - 1), skip_group_check=True)
                    w2T = small.tile([128, 512], BF, tag=f"w2T{k}")
                    if k % 2 == 0:
                        nc.vector.tensor_copy(out=w2T, in_=pt)
                    else:
                        nc.scalar.copy(out=w2T, in_=pt)
                    w2Ts.append(w2T)
                # total bias: btot = b_out + wo @ (att @ b_v)
                btot = []
                for m in range(CK):
                    pc2 = ps_b.tile([128, 1], F32, tag="pb")
                    for j in range(CK):
                        nc.tensor.matmul(
                            pc2, wo[j][:, m * 128:(m + 1) * 128], csb[j],
                            start=(j == 0), stop=(j == CK - 1))
                    bt = small.tile([128, 1], F32, tag=f"bt{m}")
                    nc.vector.tensor_add(out=bt, in0=pc2, in1=bo_sb[m])
                    btot.append(bt)
                return w2Ts, btot

            def xr_fetch(b, t):
                xrs = []
                hsl = slice(t * 512, (t + 1) * 512)
                for oc in range(CK):
                    xr = mid.tile([128, 512], F32, tag="xr", bufs=8)
                    nc.gpsimd.dma_start(
                        out=xr,
                        in_=x_d.ap()[b, oc * 128:(oc + 1) * 128, hsl])
                    xrs.append(xr)
                return xrs

            def att_block(b, t, w2Ts, btot, hp, hs, xrs, final):
                # one output t-block of the fused apply; xrs were prefetched
                hsl = slice(t * 512, (t + 1) * 512)
                for oc in range(CK):
                    po = ps_a.tile([128, 512], F32, tag="pa")
                    for k in range(CK):
                        nc.tensor.matmul(
                            po, w2Ts[k][:, oc * 128:(oc + 1) * 128],
                            h_src(hp, hs, k, t * 512, (t + 1) * 512),
                            start=(k == 0), stop=(k == CK - 1))
                    fin = mid.tile([128, 512], F32, tag="fin")
                    nc.vector.scalar_tensor_tensor(
                        out=fin, in0=po, scalar=btot[oc], in1=xrs[oc],
                        op0=OP.add, op1=OP.add)
                    eng = nc.scalar if (final and oc % 2 == 1) else nc.sync
                    eng.dma_start(
                        out=out_d.ap()[b, oc * 128:(oc + 1) * 128, hsl],
                        in_=fin)

            def att_apply(b, w2Ts, btot, hp, hs, final, extras=None):
                xrs_cur = xr_fetch(b, 0)
                for t in range(NT):
                    xrs_next = xr_fetch(b, t + 1) if t < NT - 1 else None
                    att_block(b, t, w2Ts, btot, hp, hs, xrs_cur, final)
                    if extras is not None:
                        for op in extras[t * 2:(t + 1) * 2]:
                            op()
                    xrs_cur = xrs_next

            def warm_mms(n, fp32_src=None):
                # keep the PE p-state warm with throwaway matmuls
                for i in range(n):
                    wt = ps_b.tile([128, 512], F32, tag="pb")
                    if fp32_src is None:
                        nc.tensor.matmul(wt, wqk[i % CK][:, 0:128],
                                         wqk[(i + 1) % CK][:, 0:512],
                                         start=True, stop=True)
                    else:
                        nc.tensor.matmul(wt, fp32_src[:, 0:128],
                                         fp32_src[:, 0:512],
                                         start=True, stop=True)

            def qkv_loop(hp, hs, Tsc, s0, s1, pend, hooks=None):
                # qk chunks with LAG-deferred score emission; optional
                # per-chunk hook for cross-batch overlap work
                for s in range(s0, s1):
                    pend.append(qk_chunk(hp, hs, s))
                    if len(pend) > LAG:
                        emit_scores(pend.pop(0), Tsc)
                    if hooks is not None:
                        hooks(s)

            # ================= pipeline (B=2 hardcoded shape) ==============
            assert B == 2
            xs0 = load_x(0)
            st0 = bn_alloc()
            for k in range(CK):
                for j in range(SUB):
                    bn_piece(xs0, st0, k, j)
            co0 = bn_finish(st0)
            hp0 = hpre_alloc(0)
            normalize_pre(hp0, xs0, co0)
            hs0 = h_alloc(0)
            Tsc0 = scores_setup()

            # batch 1 overlap state
            ov = {"xs1": None, "st1": None, "co1": None}
            hp1 = hpre_alloc(1)

            # piece-major bn order matched to x1 piece arrival so a DVE
            # bn_stats op never head-of-line blocks later qk evacuations
            BN_ORDER = [(k, 2 * p + jj)
                        for p in range(NP) for k in range(CK)
                        for jj in range(2)]

            def qkv0_hooks(s):
                if s == 0:
                    # x1 piece DMAs queue up; per-piece tag recycling lets
                    # each start as soon as batch 0 is done with that piece
                    ov["xs1"] = load_x(1)
                    ov["st1"] = bn_alloc()
                if s % 4 == 0 and s <= 20:
                    normalize_block(hs0, xs0, co0, s // 4 + 2)
                if 5 <= s < 29:
                    i0 = ((s - 5) * 32) // 24
                    i1 = ((s - 4) * 32) // 24
                    for pc in range(i0, i1):
                        k, j = BN_ORDER[pc]
                        bn_piece(ov["xs1"], ov["st1"], k, j)
                if s == 29:
                    ov["co1"] = bn_finish(ov["st1"])
                    ov["pre1"] = hpre_ops(hp1, ov["xs1"], ov["co1"])
                if s in (30, 31):
                    # hpre block 0 (4 ops) lands before qkv(0) drains, so
                    # the stash chunks can start the instant it ends
                    for op in ov["pre1"][(s - 30) * 2:(s - 29) * 2]:
                        op()

            pend0 = []
            qkv_loop(hp0, hs0, Tsc0, 0, SP, pend0, qkv0_hooks)
            for qk in pend0:
                emit_scores(qk, Tsc0)

            # stash: batch-1 qk chunks (reading hpre only) fill the PE
            # while batch-0 softmax + composite-weight chain runs
            hs1 = h_alloc(1)
            stash = [qk_chunk(hp1, None, s) for s in range(PRE)]
            aT0, csb0 = softmax(Tsc0)
            Tsc1 = scores_setup()
            for qk in stash:
                emit_scores(qk, Tsc1)
            wT0, bt0 = w_chain(aT0, csb0)
            # hpre(1) block-1 ops ride early in qkv(1) on ACT
            for op in ov["pre1"][4:]:
                op()

            # qkv(1) with batch-0's fused-apply t-blocks interleaved: no
            # standalone att(0) phase, so PE never waits on its evac chain
            st0_state = {"xrs": xr_fetch(0, 0), "t": 0}

            def qkv1_hooks(s):
                if s == PRE:
                    normalize_block(hs1, ov["xs1"], ov["co1"], 2)
                elif s % 4 == 0 and 4 <= s <= 20:
                    normalize_block(hs1, ov["xs1"], ov["co1"], s // 4 + 2)
                if s >= 5 and (s - 5) % 4 == 0 and st0_state["t"] < NT - 1:
                    t = st0_state["t"]
                    xrs_next = xr_fetch(0, t + 1)
                    att_block(0, t, wT0, bt0, hp0, hs0, st0_state["xrs"],
                              final=False)
                    st0_state["xrs"] = xrs_next
                    st0_state["t"] = t + 1

            pend1 = []
            qkv_loop(hp1, hs1, Tsc1, PRE, SP, pend1, qkv1_hooks)
            for qk in pend1:
                emit_scores(qk, Tsc1)
            att_block(0, NT - 1, wT0, bt0, hp0, hs0, st0_state["xrs"],
                      final=False)

            # att(1)
            aT1, csb1 = softmax(Tsc1)
            wT1, bt1 = w_chain(aT1, csb1)
            att_apply(1, wT1, bt1, hp1, hs1, final=True)

    nc.compile()
    return nc


def make_indicators():
    """Host-built groupnorm reduce/broadcast indicator matrices."""
    ch = np.arange(C)
    grp = ch // (C // G)
    indf = np.zeros((C, G), np.float32)
    indf[ch, grp] = 1.0 / (C // G)
    indb = np.zeros((G, C), np.float32)
    indb[grp, ch] = 1.0
    return indf, indb


_PROGRAM = None


def _get_program():
    global _PROGRAM
    if _PROGRAM is None:
        _PROGRAM = build_program()
    return _PROGRAM


def kernel(x, gamma, beta, w_qkv, b_qkv, w_out, b_out):
    x = np.asarray(x)
    B, C_, H, W = x.shape
    N = H * W
    assert C_ == C and B == 16 and N == 4096
    nc = _get_program()

    bf = ml_dtypes.bfloat16
    w_qkv = np.asarray(w_qkv, dtype=np.float32)
    wqkT = np.ascontiguousarray(w_qkv[:2 * C].T).astype(bf)
    wvR = np.ascontiguousarray(w_qkv[2 * C:]).astype(bf)
    woT = np.ascontiguousarray(np.asarray(w_out, dtype=np.float32).T).astype(bf)
    b_qkv = np.asarray(b_qkv, dtype=np.float32)
    bqk = np.ascontiguousarray(b_qkv[:2 * C].reshape(1, -1)).astype(bf)
    bv = np.ascontiguousarray(b_qkv[2 * C:].reshape(-1, 1)).astype(bf)
    bo = np.ascontiguousarray(np.asarray(b_out, np.float32).reshape(-1, 1))
    gam = np.ascontiguousarray(np.asarray(gamma, np.float32).reshape(-1, 1))
    bet = np.ascontiguousarray(np.asarray(beta, np.float32).reshape(-1, 1))
    xr = np.ascontiguousarray(x.reshape(B, C, N).astype(np.float32))

    indf, indb = make_indicators()
    bpc = B // N_CORES
    in_maps = []
    for c in range(N_CORES):
        in_maps.append({
            "x": xr[c * bpc:(c + 1) * bpc],
            "wqkT": wqkT, "wvR": wvR, "woT": woT,
            "bqk": bqk, "bv": bv, "bo": bo,
            "gamma": gam, "beta": bet,
            "indf": indf, "indb": indb,
        })
    res = run_bass_kernel_spmd(nc, in_maps, core_ids=list(range(N_CORES)))
    out = np.concatenate([res.results[c]["out"] for c in range(N_CORES)],
                         axis=0)
    return out.reshape(B, C_, H, W).astype(np.float32)


# revision 19
# speedup vs baseline: 1.1543x; 1.1543x over previous
"""Trainium2 Bass kernel for nn_AttentionBlock (B=16, C=512, H=W=64, 8 heads).

Channel-attention block: GroupNorm(8 groups) -> 1x1 qkv -> scores over
channel dims (contract spatial N=4096) -> softmax -> att @ v -> 1x1 out
projection -> residual.

Sharding: data-parallel over batch. 16 batches / 8 cores = 2 per core.
No collectives. Each core runs the identical program on its 2 batches.

Key algebraic optimization: since the attention matrix is tiny (eight
64x64 blocks) and is applied over N=4096 spatial positions, the output
projection is reassociated as

    out = (w_out @ blockdiag(att) @ w_v) @ h + (w_out @ (att@b_v) + b_out) + x

so the v-projection, att@v, and both of their PSUM evacuations collapse
into one small per-batch composite-weight build (w' = att@wv, w'' = wo@w',
transpose) plus a single [C,C]@[C,N] apply pass.

Layouts / engine split:
  x      [C, N] fp32 as 16 per-piece tiles (piece-level recycling lets
         batch b+1's loads start as soon as batch b frees each piece)
  h      bf16; blocks 0-1 in double-buffered hpre tiles, blocks 2-7 in
         double-buffered [128, 3072] tiles (lazy ACT normalize)
  q,k    [N, 2C] bf16; psum evac: q-half fused bias-add on DVE,
         k-half ACT copy + Pool bias-add; scores deferred LAG chunks
  scores head-pair matmuls (128-wide stationary, 2 heads) into one
         [128,512] psum tile; off-diagonal quadrants are ignored garbage
  attT   4 block-diagonal [128,128] bf16 tiles
  bn     bn_stats on DVE, spread through the qk chunk loop piece-major,
         matched to x-piece arrival (no DVE head-of-line blocking)

Software pipeline: batch 1's x load + groupnorm stats + hpre normalize
ride inside batch 0's qk loop; batch 0's fused-apply t-blocks (+ xr
residual prefetch on the SWDGE ring) are interleaved into batch 1's qk
chunk loop, so there is no standalone att(0) phase. A 3-chunk qk stash
covers the softmax+weight-chain latency. Residual adds are DVE STTs;
stores go out on the sync (and scalar, for the last batch) HWDGE rings.
"""

import numpy as np
import ml_dtypes

import concourse.bacc as bacc
import concourse.tile as tile
from concourse import mybir
from concourse.bass_utils import run_bass_kernel_spmd
from concourse.masks import make_identity

BF = mybir.dt.bfloat16
F32 = mybir.dt.float32
AX = mybir.AxisListType
OP = mybir.AluOpType
AF = mybir.ActivationFunctionType

C = 512
NH = 8
D = 64  # head dim
G = 8   # groupnorm groups
CK = C // 128  # 4 channel chunks
EPS = 1e-5
N_CORES = 8


def build_program(B=2, N=4096, debug=False):
    SP = N // 128   # spatial chunks for qk/scores
    NT = N // 512   # 512-col tiles
    SUB = N // 512  # bn_stats subgroups (free dim <= 512)
    scale = float(1.0 / np.sqrt(D))
    LAG = 3         # score-emission lag behind qk chunks (ACT->Pool evac)
    PRE = 3         # batch-(b+1) qk chunks stashed during batch-b softmax

    nc = bacc.Bacc("TRN2", target_bir_lowering=False, debug=debug,
                   num_devices=N_CORES)

    x_d = nc.dram_tensor("x", [B, C, N], F32, kind="ExternalInput")
    wqk_d = nc.dram_tensor("wqkT", [C, 2 * C], BF, kind="ExternalInput")
    wv_d = nc.dram_tensor("wvR", [C, C], BF, kind="ExternalInput")
    wo_d = nc.dram_tensor("woT", [C, C], BF, kind="ExternalInput")
    bqk_d = nc.dram_tensor("bqk", [1, 2 * C], BF, kind="ExternalInput")
    bv_d = nc.dram_tensor("bv", [C, 1], BF, kind="ExternalInput")
    bo_d = nc.dram_tensor("bo", [C, 1], F32, kind="ExternalInput")
    gam_d = nc.dram_tensor("gamma", [C, 1], F32, kind="ExternalInput")
    bet_d = nc.dram_tensor("beta", [C, 1], F32, kind="ExternalInput")
    indf_d = nc.dram_tensor("indf", [C, G], F32, kind="ExternalInput")
    indb_d = nc.dram_tensor("indb", [G, C], F32, kind="ExternalInput")
    out_d = nc.dram_tensor("out", [B, C, N], F32, kind="ExternalOutput")

    with tile.TileContext(nc) as tc:
        import contextlib
        ctx = contextlib.ExitStack()
        with ctx:
            persist = ctx.enter_context(tc.tile_pool(name="persist", bufs=1))
            big = ctx.enter_context(tc.tile_pool(name="big", bufs=1))
            mid = ctx.enter_context(tc.tile_pool(name="mid", bufs=3))
            small = ctx.enter_context(tc.tile_pool(name="small", bufs=1))
            # pool A: qk projection halves (qkv phase) + out-proj (att phase)
            # pool B: v-proj, hv, transposes, groupnorm matmuls
            ps_a = ctx.enter_context(
                tc.tile_pool(name="ps_a", bufs=4, space="PSUM"))
            ps_sc = ctx.enter_context(
                tc.tile_pool(name="ps_sc", bufs=1, space="PSUM"))
            ps_b = ctx.enter_context(
                tc.tile_pool(name="ps_b", bufs=3, space="PSUM"))

            # ---- persistent: weights / constants ----
            wqk = []
            wv = []
            wo = []
            bv_sb = []
            bo_sb = []
            gam = []
            bet = []
            for k in range(CK):
                t = persist.tile([128, 2 * C], BF, tag=f"wqk{k}")
                nc.gpsimd.dma_start(out=t, in_=wqk_d.ap()[k * 128:(k + 1) * 128, :])
                wqk.append(t)
                t = persist.tile([128, C], BF, tag=f"wv{k}")
                nc.gpsimd.dma_start(out=t, in_=wv_d.ap()[k * 128:(k + 1) * 128, :])
                wv.append(t)  # raw wv rows [e-chunk, cin] for w' = att @ wv
                t = persist.tile([128, C], BF, tag=f"wo{k}")
                nc.gpsimd.dma_start(out=t, in_=wo_d.ap()[k * 128:(k + 1) * 128, :])
                wo.append(t)
                t = persist.tile([128, 1], BF, tag=f"bv{k}")
                nc.gpsimd.dma_start(out=t, in_=bv_d.ap()[k * 128:(k + 1) * 128, :])
                bv_sb.append(t)
                t = persist.tile([128, 1], F32, tag=f"bo{k}")
                nc.gpsimd.dma_start(out=t, in_=bo_d.ap()[k * 128:(k + 1) * 128, :])
                bo_sb.append(t)
                t = persist.tile([128, 1], F32, tag=f"gam{k}")
                nc.gpsimd.dma_start(out=t, in_=gam_d.ap()[k * 128:(k + 1) * 128, :])
                gam.append(t)
                t = persist.tile([128, 1], F32, tag=f"bet{k}")
                nc.gpsimd.dma_start(out=t, in_=bet_d.ap()[k * 128:(k + 1) * 128, :])
                bet.append(t)
            # q/k bias replicated across all 128 partitions (spatial rows)
            import concourse.bass as bass
            bqk_rep = persist.tile([128, 2 * C], BF, tag="bqk_rep")
            _bqk_ap = bqk_d.ap()
            nc.gpsimd.dma_start(
                out=bqk_rep,
                in_=bass.AP(tensor=_bqk_ap.tensor, offset=_bqk_ap.offset,
                            ap=[[0, 128], [1, 2 * C]]))

            zero1 = persist.tile([1, 128], BF, tag="zero1")
            nc.gpsimd.memset(zero1, 0.0)
            zrhs512 = persist.tile([1, 512], BF, tag="zrhs512")
            nc.gpsimd.memset(zrhs512, 0.0)
            ident = persist.tile([128, 128], BF, tag="ident")
            make_identity(nc, ident)
            eps_t = persist.tile([128, 1], F32, tag="eps")
            nc.gpsimd.memset(eps_t, EPS)
            # block-diagonal att tiles: off-diagonal quadrants stay zero
            att_bf = []
            for p in range(CK):
                t = persist.tile([128, 128], BF, tag=f"attbf{p}")
                nc.gpsimd.memset(t, 0.0)
                att_bf.append(t)
            # group indicator matrices (groupnorm cross-partition reduce)
            indf = []
            for k in range(CK):
                t = persist.tile([128, G], F32, tag=f"indf{k}")
                nc.gpsimd.dma_start(
                    out=t, in_=indf_d.ap()[k * 128:(k + 1) * 128, :])
                indf.append(t)
            indb = persist.tile([G, C], F32, tag="indb")
            nc.gpsimd.dma_start(out=indb, in_=indb_d.ap())

            # ---- building blocks ----
            NP = N // 1024  # x pieces per chunk

            def load_x(b):
                # per-piece tiles so batch b+1 pieces can recycle as soon as
                # the prior batch's readers of that piece are done; DMAs are
                # emitted piece-major so a late piece never head-of-line
                # blocks another chunk's earlier piece on the sync ring
                xs = [[None] * NP for _ in range(CK)]
                for p in range(NP):
                    for k in range(CK):
                        t = big.tile([128, 1024], F32, tag=f"x{k}p{p}",
                                     name=f"x{k}p{p}")
                        nc.sync.dma_start(
                            out=t,
                            in_=x_d.ap()[b, k * 128:(k + 1) * 128,
                                         p * 1024:(p + 1) * 1024])
                        xs[k][p] = t
                return xs

            def bn_alloc():
                sts = []
                for k in range(CK):
                    sts.append(small.tile([128, SUB, 6], F32, tag=f"st{k}",
                                          name=f"st{k}"))
                return sts

            def bn_piece(xs, sts, k, j):
                nc.vector.bn_stats(
                    out=sts[k][:, j, :],
                    in_=xs[k][j // 2][:, (j % 2) * 512:(j % 2) * 512 + 512])

            def bn_finish(sts):
                mvs = []
                for k in range(CK):
                    mv = small.tile([128, 2], F32, tag=f"mv{k}")
                    nc.vector.bn_aggr(out=mv, in_=sts[k])
                    mvs.append(mv)
                # rhs2: col0 = mean_p, col1 = mean_p^2 + var_p = E[x^2]_p
                rhs2s = []
                for k in range(CK):
                    r2 = small.tile([128, 2], F32, tag=f"r2{k}")
                    nc.gpsimd.tensor_copy(out=r2[:, 0:1], in_=mvs[k][:, 0:1])
                    nc.vector.scalar_tensor_tensor(
                        out=r2[:, 1:2], in0=mvs[k][:, 0:1],
                        scalar=mvs[k][:, 0:1], in1=mvs[k][:, 1:2],
                        op0=OP.mult, op1=OP.add)
                    rhs2s.append(r2)
                # cross-partition reduce to per-group stats [8, 2]
                pg = ps_b.tile([G, 2], F32, tag="pb")
                for k in range(CK):
                    nc.tensor.matmul(pg, indf[k], rhs2s[k],
                                     start=(k == 0), stop=(k == CK - 1))
                sg = small.tile([G, 2], F32, tag="sg")
                nc.vector.tensor_copy(out=sg, in_=pg)
                t2 = small.tile([G, 1], F32, tag="t2")
                nc.vector.tensor_mul(out=t2, in0=sg[:, 0:1], in1=sg[:, 0:1])
                vs = small.tile([G, 1], F32, tag="vs")
                nc.vector.tensor_sub(out=vs, in0=sg[:, 1:2], in1=t2)
                # rstd = exp(-0.5 * ln(var + eps)); Ln/Exp share a table set
                lnv = small.tile([G, 1], F32, tag="lnv")
                nc.scalar.activation(out=lnv, in_=vs, func=AF.Ln,
                                     bias=eps_t[0:G, :], scale=1.0)
                rstd = small.tile([G, 1], F32, tag="rstd")
                nc.scalar.activation(out=rstd, in_=lnv, func=AF.Exp, scale=-0.5)
                bcr = small.tile([G, 2], F32, tag="bcr")
                nc.gpsimd.tensor_copy(out=bcr[:, 0:1], in_=sg[:, 0:1])
                nc.gpsimd.tensor_copy(out=bcr[:, 1:2], in_=rstd)
                # broadcast group stats back to channels; affine coeffs
                scs = []
                nbs = []
                for k in range(CK):
                    pbc = ps_b.tile([128, 2], F32, tag="pb")
                    nc.tensor.matmul(pbc, indb[:, k * 128:(k + 1) * 128], bcr,
                                     start=True, stop=True)
                    sc = small.tile([128, 1], F32, tag=f"sc{k}")
                    nc.vector.tensor_mul(out=sc, in0=pbc[:, 1:2], in1=gam[k])
                    t4 = small.tile([128, 1], F32, tag=f"t4{k}")
                    nc.vector.tensor_scalar_mul(out=t4, in0=pbc[:, 0:1],
                                                scalar1=sc)
                    nb = small.tile([128, 1], F32, tag=f"nb{k}")
                    nc.vector.tensor_sub(out=nb, in0=bet[k], in1=t4)
                    scs.append(sc)
                    nbs.append(nb)
                return scs, nbs

            def h_alloc(par):
                # blocks 2..7 only (cols 1024:4096); blocks 0-1 live in hpre
                return [big.tile([128, N - 1024], BF, tag=f"h{k}b{par}",
                                 name=f"h{k}")
                        for k in range(CK)]

            def normalize_block(hs, xs, coeffs, t):
                # ACT: out = Identity(scale*x + bias), per-partition APs.
                # h blocks 0-1 live in the small hpre tiles (cols 0:1024);
                # hs tiles carry cols 1024:4096 only (offset by 1024).
                scs, nbs = coeffs
                xsl = slice((t % 2) * 512, (t % 2) * 512 + 512)
                sl = slice(t * 512 - 1024, (t + 1) * 512 - 1024)
                for k in range(CK):
                    nc.scalar.activation(
                        out=hs[k][:, sl], in_=xs[k][t // 2][:, xsl],
                        func=AF.Identity, bias=nbs[k], scale=scs[k])

            def hpre_alloc(par):
                return [big.tile([128, 1024], BF, tag=f"hpre{k}b{par}",
                                 name=f"hpre{k}")
                        for k in range(CK)]

            def hpre_ops(hp, xs, coeffs):
                # 8 thunks: normalize blocks 0-1 into the hpre tiles
                scs, nbs = coeffs
                ops = []
                for t in range(2):
                    xsl = slice((t % 2) * 512, (t % 2) * 512 + 512)
                    sl = slice(t * 512, (t + 1) * 512)
                    for k in range(CK):
                        def op(t=t, k=k, xsl=xsl, sl=sl):
                            nc.scalar.activation(
                                out=hp[k][:, sl], in_=xs[k][0][:, xsl],
                                func=AF.Identity, bias=nbs[k], scale=scs[k])
                        ops.append(op)
                return ops

            def normalize_pre(hp, xs, coeffs):
                for op in hpre_ops(hp, xs, coeffs):
                    op()

            def h_src(hp, hs, k, c0, c1):
                if c1 <= 1024:
                    return hp[k][:, c0:c1]
                return hs[k][:, c0 - 1024:c1 - 1024]

            def scores_setup():
                Tsc = ps_sc.tile([128, 512], F32, tag="sc")
                nc.tensor.matmul(Tsc, zero1, zrhs512, start=True, stop=False,
                                 skip_group_check=True)
                return Tsc

            def v_alloc():
                return [big.tile([128, N], BF, tag=f"v{k}", name=f"v{k}")
                        for k in range(CK)]

            def qk_chunk(hp, hs, s):
                # qk projection for one 128-row spatial chunk; evac on ACT,
                # bias add on Pool (scores consume LAG chunks later)
                qk = mid.tile([128, 2 * C], BF, tag="qk", bufs=5)
                for half in range(2):
                    pqk = ps_a.tile([128, 512], F32, tag="pa")
                    wseg = slice(half * 512, (half + 1) * 512)
                    for k in range(CK):
                        nc.tensor.matmul(
                            pqk, h_src(hp, hs, k, s * 128, (s + 1) * 128),
                            wqk[k][:, wseg], start=(k == 0),
                            stop=(k == CK - 1))
                    if half == 0:
                        # fused evac + bias add in one DVE pass
                        nc.vector.tensor_add(out=qk[:, wseg], in0=pqk,
                                             in1=bqk_rep[:, wseg])
                    else:
                        nc.scalar.copy(out=qk[:, wseg], in_=pqk)
                        nc.gpsimd.tensor_add(out=qk[:, wseg],
                                             in0=qk[:, wseg],
                                             in1=bqk_rep[:, wseg])
                return qk

            def emit_scores(qk, Tsc):
                # head-pair matmuls: stationary = 2 heads of q (128 cols);
                # diagonal 64x64 blocks of each [128,128] slot are the real
                # per-head scores, off-diagonal is ignored garbage.
                for p in range(CK):
                    nc.tensor.matmul(
                        Tsc[:, p * 128:(p + 1) * 128],
                        qk[:, p * 128:(p + 1) * 128],
                        qk[:, 512 + p * 128:512 + (p + 1) * 128],
                        start=False, stop=False, skip_group_check=True)

            def softmax(Tsc):
                # softmax without max-subtraction: logits = S/8 are bounded
                # well inside fp32 exp range for this distribution.
                p_f = small.tile([128, 512], F32, tag="pf")
                nc.scalar.activation(out=p_f, in_=Tsc, func=AF.Exp,
                                     scale=scale)
                rsum = small.tile([128, CK], F32, tag="rsum")
                for p in range(CK):
                    c0 = p * 128
                    nc.vector.reduce_sum(
                        out=rsum[0:64, p:p + 1],
                        in_=p_f[0:64, c0:c0 + 64], axis=AX.X)
                    nc.vector.reduce_sum(
                        out=rsum[64:128, p:p + 1],
                        in_=p_f[64:128, c0 + 64:c0 + 128], axis=AX.X)
                rinv = small.tile([128, CK], F32, tag="rinv")
                nc.vector.reciprocal(out=rinv, in_=rsum)
                for p in range(CK):
                    c0 = p * 128
                    nc.vector.tensor_scalar_mul(
                        out=att_bf[p][0:64, 0:64],
                        in0=p_f[0:64, c0:c0 + 64],
                        scalar1=rinv[0:64, p:p + 1])
                    nc.vector.tensor_scalar_mul(
                        out=att_bf[p][64:128, 64:128],
                        in0=p_f[64:128, c0 + 64:c0 + 128],
                        scalar1=rinv[64:128, p:p + 1])
                aTs = []
                for p in range(CK):
                    ptr = ps_b.tile([128, 128], BF, tag="pb")
                    nc.tensor.transpose(ptr, att_bf[p], ident)
                    aT = small.tile([128, 128], BF, tag=f"aT{p}")
                    nc.vector.tensor_copy(out=aT, in_=ptr)
                    aTs.append(aT)
                # c = att @ b_v, later folded into the composite bias
                csb = []
                for k in range(CK):
                    pcv = ps_b.tile([128, 1], F32, tag="pb")
                    nc.tensor.matmul(pcv, aTs[k], bv_sb[k],
                                     start=True, stop=True)
                    ct = small.tile([128, 1], BF, tag=f"c{k}")
                    nc.vector.tensor_copy(out=ct, in_=pcv)
                    csb.append(ct)
                return aTs, csb

            def w_chain(aTs, csb):
                # Reassociated attention: out = (wo @ blockdiag(att) @ wv) @ h.
                # The composite weight is tiny ([C, C]) vs applying att over
                # N=4096 columns, so v-projection, att@v and both their
                # evacuations collapse into one weight-build + one apply pass.
                # w' = blockdiag(att) @ wv : [cout-block j, cin]
                wps = []
                for j in range(CK):
                    pw = ps_b.tile([128, 512], F32, tag="pb")
                    nc.tensor.matmul(pw, aTs[j], wv[j], start=True, stop=True)
                    wp = small.tile([128, 512], BF, tag=f"wp{j}")
                    nc.vector.tensor_copy(out=wp, in_=pw)
                    wps.append(wp)
                # w'' = wo @ w' : [oout-block m, cin]
                w2s = []
                for m in range(CK):
                    pw2 = ps_b.tile([128, 512], F32, tag="pb")
                    for j in range(CK):
                        nc.tensor.matmul(
                            pw2, wo[j][:, m * 128:(m + 1) * 128], wps[j],
                            start=(j == 0), stop=(j == CK - 1))
                    w2 = small.tile([128, 512], BF, tag=f"w2{m}")
                    nc.scalar.copy(out=w2, in_=pw2)
                    w2s.append(w2)
                # w''T for use as matmul stationary: [cin-chunk k, oout].
                # 4 transposes land in one psum tile; evacs alternate DVE/ACT
                w2Ts = []
                for k in range(CK):
                    pt = ps_b.tile([128, 512], BF, tag="pb")
                    for m in range(CK):
                        nc.tensor.matmul(
                            pt[:, m * 128:(m + 1) * 128],
                            w2s[m][:, k * 128:(k + 1) * 128], ident,
                            is_transpose=True, start=(m == 0),
                            stop=(m == CK - 1), skip_group_check=True)
                    w2T = small.tile([128, 512], BF, tag=f"w2T{k}")
                    if k % 2 == 0:
                        nc.vector.tensor_copy(out=w2T, in_=pt)
                    else:
                        nc.scalar.copy(out=w2T, in_=pt)
                    w2Ts.append(w2T)
                # total bias: btot = b_out + wo @ (att @ b_v)
                btot = []
                for m in range(CK):
                    pc2 = ps_b.tile([128, 1], F32, tag="pb")
                    for j in range(CK):
                        nc.tensor.matmul(
                            pc2, wo[j][:, m * 128:(m + 1) * 128], csb[j],
                            start=(j == 0), stop=(j == CK - 1))
                    bt = small.tile([128, 1], F32, tag=f"bt{m}")
                    nc.vector.tensor_add(out=bt, in0=pc2, in1=bo_sb[m])
                    btot.append(bt)
                return w2Ts, btot

            def xr_fetch(b, t):
                xrs = []
                hsl = slice(t * 512, (t + 1) * 512)
                for oc in range(CK):
                    xr = mid.tile([128, 512], F32, tag="xr", bufs=8)
                    nc.gpsimd.dma_start(
                        out=xr,
                        in_=x_d.ap()[b, oc * 128:(oc + 1) * 128, hsl])
                    xrs.append(xr)
                return xrs

            def att_block(b, t, w2Ts, btot, hp, hs, xrs, final):
                # one output t-block of the fused apply; xrs were prefetched
                hsl = slice(t * 512, (t + 1) * 512)
                for oc in range(CK):
                    po = ps_a.tile([128, 512], F32, tag="pa")
                    for k in range(CK):
                        nc.tensor.matmul(
                            po, w2Ts[k][:, oc * 128:(oc + 1) * 128],
                            h_src(hp, hs, k, t * 512, (t + 1) * 512),
                            start=(k == 0), stop=(k == CK - 1))
                    fin = mid.tile([128, 512], F32, tag="fin")
                    nc.vector.scalar_tensor_tensor(
                        out=fin, in0=po, scalar=btot[oc], in1=xrs[oc],
                        op0=OP.add, op1=OP.add)
                    eng = nc.scalar if (final and oc % 2 == 1) else nc.sync
                    eng.dma_start(
                        out=out_d.ap()[b, oc * 128:(oc + 1) * 128, hsl],
                        in_=fin)

            def att_apply(b, w2Ts, btot, hp, hs, final, extras=None):
                xrs_cur = xr_fetch(b, 0)
                for t in range(NT):
                    xrs_next = xr_fetch(b, t + 1) if t < NT - 1 else None
                    att_block(b, t, w2Ts, btot, hp, hs, xrs_cur, final)
                    if extras is not None:
                        for op in extras[t * 2:(t + 1) * 2]:
                            op()
                    xrs_cur = xrs_next

            def warm_mms(n, fp32_src=None):
                # keep the PE p-state warm with throwaway matmuls
                for i in range(n):
                    wt = ps_b.tile([128, 512], F32, tag="pb")
                    if fp32_src is None:
                        nc.tensor.matmul(wt, wqk[i % CK][:, 0:128],
                                         wqk[(i + 1) % CK][:, 0:512],
                                         start=True, stop=True)
                    else:
                        nc.tensor.matmul(wt, fp32_src[:, 0:128],
                                         fp32_src[:, 0:512],
                                         start=True, stop=True)

            def qkv_loop(hp, hs, Tsc, s0, s1, pend, hooks=None):
                # qk chunks with LAG-deferred score emission; optional
                # per-chunk hook for cross-batch overlap work
                for s in range(s0, s1):
                    pend.append(qk_chunk(hp, hs, s))
                    if len(pend) > LAG:
                        emit_scores(pend.pop(0), Tsc)
                    if hooks is not None:
                        hooks(s)

            # ================= pipeline (B=2 hardcoded shape) ==============
            assert B == 2
            xs0 = load_x(0)
            st0 = bn_alloc()
            for k in range(CK):
                for j in range(SUB):
                    bn_piece(xs0, st0, k, j)
            co0 = bn_finish(st0)
            hp0 = hpre_alloc(0)
            normalize_pre(hp0, xs0, co0)
            hs0 = h_alloc(0)
            Tsc0 = scores_setup()

            # batch 1 overlap state
            ov = {"xs1": None, "st1": None, "co1": None}
            hp1 = hpre_alloc(1)

            # piece-major bn order matched to x1 piece arrival so a DVE
            # bn_stats op never head-of-line blocks later qk evacuations
            BN_ORDER = [(k, 2 * p + jj)
                        for p in range(NP) for k in range(CK)
                        for jj in range(2)]

            def qkv0_hooks(s):
                if s == 0:
                    # x1 piece DMAs queue up; per-piece tag recycling lets
                    # each start as soon as batch 0 is done with that piece
                    ov["xs1"] = load_x(1)
                    ov["st1"] = bn_alloc()
                if s % 4 == 0 and s <= 20:
                    normalize_block(hs0, xs0, co0, s // 4 + 2)
                if 5 <= s < 29:
                    i0 = ((s - 5) * 32) // 24
                    i1 = ((s - 4) * 32) // 24
                    for pc in range(i0, i1):
                        k, j = BN_ORDER[pc]
                        bn_piece(ov["xs1"], ov["st1"], k, j)
                if s == 29:
                    ov["co1"] = bn_finish(ov["st1"])
                    ov["pre1"] = hpre_ops(hp1, ov["xs1"], ov["co1"])
                if s in (30, 31):
                    # hpre block 0 (4 ops) lands before qkv(0) drains, so
                    # the stash chunks can start the instant it ends
                    for op in ov["pre1"][(s - 30) * 2:(s - 29) * 2]:
                        op()

            pend0 = []
            qkv_loop(hp0, hs0, Tsc0, 0, SP, pend0, qkv0_hooks)
            for qk in pend0:
                emit_scores(qk, Tsc0)

            # stash: batch-1 qk chunks (reading hpre only) fill the PE
            # while batch-0 softmax + composite-weight chain runs
            hs1 = h_alloc(1)
            stash = [qk_chunk(hp1, None, s) for s in range(PRE)]
            aT0, csb0 = softmax(Tsc0)
            Tsc1 = scores_setup()
            for qk in stash:
                emit_scores(qk, Tsc1)
            wT0, bt0 = w_chain(aT0, csb0)
            # hpre(1) block-1 ops ride early in qkv(1) on ACT
            for op in ov["pre1"][4:]:
                op()

            # qkv(1) with batch-0's fused-apply t-blocks interleaved: no
            # standalone att(0) phase, so PE never waits on its evac chain
            st0_state = {"xrs": xr_fetch(0, 0), "t": 0}

            def qkv1_hooks(s):
                if s == PRE:
                    normalize_block(hs1, ov["xs1"], ov["co1"], 2)
                elif s % 4 == 0 and 4 <= s <= 20:
                    normalize_block(hs1, ov["xs1"], ov["co1"], s // 4 + 2)
                if s >= 5 and (s - 5) % 4 == 0 and st0_state["t"] < NT - 1:
                    t = st0_state["t"]
                    xrs_next = xr_fetch(0, t + 1)
                    att_block(0, t, wT0, bt0, hp0, hs0, st0_state["xrs"],
                              final=False)
                    st0_state["xrs"] = xrs_next
                    st0_state["t"] = t + 1

            pend1 = []
            qkv_loop(hp1, hs1, Tsc1, PRE, SP, pend1, qkv1_hooks)
            for qk in pend1:
                emit_scores(qk, Tsc1)
            att_block(0, NT - 1, wT0, bt0, hp0, hs0, st0_state["xrs"],
                      final=False)

            # att(1)
            aT1, csb1 = softmax(Tsc1)
            wT1, bt1 = w_chain(aT1, csb1)
            att_apply(1, wT1, bt1, hp1, hs1, final=True)

    nc.compile()
    return nc


def make_indicators():
    """Host-built groupnorm reduce/broadcast indicator matrices."""
    ch = np.arange(C)
    grp = ch // (C // G)
    indf = np.zeros((C, G), np.float32)
    indf[ch, grp] = 1.0 / (C // G)
    indb = np.zeros((G, C), np.float32)
    indb[grp, ch] = 1.0
    return indf, indb


_PROGRAM = None


def _get_program():
    global _PROGRAM
    if _PROGRAM is None:
        _PROGRAM = build_program()
    return _PROGRAM


def kernel(x, gamma, beta, w_qkv, b_qkv, w_out, b_out):
    x = np.asarray(x)
    B, C_, H, W = x.shape
    N = H * W
    assert C_ == C and B == 16 and N == 4096
    nc = _get_program()

    bf = ml_dtypes.bfloat16
    w_qkv = np.asarray(w_qkv, dtype=np.float32)
    wqkT = np.ascontiguousarray(w_qkv[:2 * C].T).astype(bf)
    wvR = np.ascontiguousarray(w_qkv[2 * C:]).astype(bf)
    woT = np.ascontiguousarray(np.asarray(w_out, dtype=np.float32).T).astype(bf)
    b_qkv = np.asarray(b_qkv, dtype=np.float32)
    bqk = np.ascontiguousarray(b_qkv[:2 * C].reshape(1, -1)).astype(bf)
    bv = np.ascontiguousarray(b_qkv[2 * C:].reshape(-1, 1)).astype(bf)
    bo = np.ascontiguousarray(np.asarray(b_out, np.float32).reshape(-1, 1))
    gam = np.ascontiguousarray(np.asarray(gamma, np.float32).reshape(-1, 1))
    bet = np.ascontiguousarray(np.asarray(beta, np.float32).reshape(-1, 1))
    xr = np.ascontiguousarray(x.reshape(B, C, N).astype(np.float32))

    indf, indb = make_indicators()
    bpc = B // N_CORES
    in_maps = []
    for c in range(N_CORES):
        in_maps.append({
            "x": xr[c * bpc:(c + 1) * bpc],
            "wqkT": wqkT, "wvR": wvR, "woT": woT,
            "bqk": bqk, "bv": bv, "bo": bo,
            "gamma": gam, "beta": bet,
            "indf": indf, "indb": indb,
        })
    res = run_bass_kernel_spmd(nc, in_maps, core_ids=list(range(N_CORES)))
    out = np.concatenate([res.results[c]["out"] for c in range(N_CORES)],
                         axis=0)
    return out.reshape(B, C_, H, W).astype(np.float32)
